# revision 3
# baseline (speedup 1.0000x reference)
"""Trainium2 Bass kernel for nn_MultiHeadAttention_78864189489198.

Reference (per batch b): q = x @ Wq; qh = heads(q); S = qh qh^T / sqrt(HD)
(SYMMETRIC since q=k=v); attn = softmax(S); y = attn @ qh;
out = merge(y) @ Wo + bo.

Sharding (8 cores): core = b*4 + hg handles batch b and head-group hg
(4 heads = 512 Wq columns / Wo rows). Host sums the 4 partial outputs
per batch and adds bo (the tensor-parallel all-reduce, host-side since
I/O is host-side anyway).

Per-core dataflow (fp16 operands everywhere, fp32 psum accumulation):
  QT[hd, tok] = Wq-slice^T @ x^T chunks            (fp16 matmuls)
  Qnat = XBAR DMA transpose of QT                  (zero PE cost)
  scores: only the UPPER TRIANGLE of S is computed (S symmetric);
    exp via ACT (scale, bias=-C) -> es fp16, accum_out = partial rowsum
  lower triangle of exp(S) reconstructed via XBAR DMA transposes of the
    exp'd upper strips (zero PE/ACT cost; one transpose per strip)
  missing rowsum parts (colsum of upper strip == rowsum of transposed
    region) via DVE tensor_reduce over the reconstructed left parts
  PV: YT_unnorm[hd, tok] = sum_a Qnat_a^T @ es_a   (fp16)
  evac+normalize fused: yt = psum * (1/r broadcast row)  (one DVE op)
  out-proj: out[tok, :] = sum_c YT_c^T @ Wo_c      (fp16), fp16 output

Pipeline: stage A = loads + Qproj + scores/exp of heads 0,1 (c2/c3
chains as PE filler).  Then iters: A' PV(0)+scores(2), B' PV(1)+
scores(3), C' PV(2), D' PV(3), out-proj.  ACT never gates the tail.
"""

import os

import numpy as np

import concourse.bass as bass
import concourse.mybir as mybir
import concourse.tile as tile
from concourse import bacc
from concourse.bass_utils import run_bass_kernel_spmd
from contextlib import ExitStack

P = 128
N = 2048          # tokens
D = 2048          # model dim
KO = D // P       # 16 contraction chunks
HG = 4            # heads per core
HD = 128          # head dim
HCOLS = HG * HD   # 512 q-columns per core
NCH = N // P      # 16 token chunks
NSP = 4           # 512-token spans
SCALE = HD ** -0.5
C_BIAS = 11.0     # exp(S*SCALE - C): keeps fp16 es AND fp16 unnormalized
                  # YT in range (max scaled S ~19 -> es_max ~ e^8 = 3e3,
                  # |YT_unnorm| < ~2e4 < 65504)

f32 = mybir.dt.float32
f16 = mybir.dt.float16

_CACHE = {}


def build_nc():
    nc = bacc.Bacc("TRN2", target_bir_lowering=False, debug=False)
    xt = nc.dram_tensor("xt", [D, N], f16, kind="ExternalInput")
    wq = nc.dram_tensor("wq", [D, HCOLS], f16, kind="ExternalInput")
    wo = nc.dram_tensor("wo", [HCOLS, D], f16, kind="ExternalInput")
    out = nc.dram_tensor("out", [N, D], f16, kind="ExternalOutput")

    xt3 = xt.rearrange("(ko p) n -> p ko n", p=P)      # [128, 16, 2048]
    wq3 = wq.rearrange("(ko p) m -> p ko m", p=P)      # [128, 16, 512]
    wo3 = wo.rearrange("(c p) n -> p c n", p=P)        # [128, 4, 2048]
    out3 = out.rearrange("(a p) n -> p a n", p=P)      # [128, 16, 2048]

    with (
        nc.allow_low_precision(reason="fp16 intermediates are intentional"),
        tile.TileContext(nc) as tc,
        ExitStack() as ctx,
    ):
        const_pool = ctx.enter_context(tc.tile_pool(name="const", bufs=1))
        qt_pool = ctx.enter_context(tc.tile_pool(name="qt", bufs=1))
        es_pool = ctx.enter_context(tc.tile_pool(name="es", bufs=2))
        rr_pool = ctx.enter_context(tc.tile_pool(name="rr", bufs=2))
        rr2_pool = ctx.enter_context(tc.tile_pool(name="rr2", bufs=2))
        rt_pool = ctx.enter_context(tc.tile_pool(name="rt", bufs=1))
        ps_s = ctx.enter_context(tc.tile_pool(name="ps_s", bufs=2, space="PSUM"))
        ps_b = ctx.enter_context(tc.tile_pool(name="ps_b", bufs=4, space="PSUM"))

        cbias = const_pool.tile([P, 1], f32, tag="cbias")
        nc.gpsimd.memset(cbias[:], -C_BIAS)
        qt_sb = qt_pool.tile([P, HG, N], f16, tag="qt")     # QT: [hd, c, tok]
        rrec2 = rt_pool.tile([P, P], f16, tag="rrec2")      # 1/r in cols 0:16
        rt = rt_pool.tile([P, P], f16, tag="rt")            # transposed 1/r
        nc.gpsimd.memset(rrec2[:], 0.0)

        # preload the exp table so the first real exp doesn't pay the load
        edum = const_pool.tile([P, 1], f32, tag="edum")
        nc.scalar.activation(edum[:], cbias[:], mybir.ActivationFunctionType.Exp)

        # ---------------- helpers ----------------
        def scores_chunk(c, a, es_t, rrech):
            """Upper-triangle scores row-chunk a of head c: matmuls into
            psum, exp (+rowsum accum) into es columns [a*128, 2048)."""
            base = a * P
            slot = 0
            off = base
            while off < N:
                w = min(1024, N - off)
                ps = ps_s.tile([P, 1024], f32, tag="s")
                o = 0
                while o < w:
                    mw = min(512, w - o)
                    nc.tensor.matmul(
                        ps[:, o:o + mw],
                        qt_sb[:, c, base:base + P],
                        qt_sb[:, c, off + o:off + o + mw],
                        start=True,
                        stop=True,
                    )
                    o += mw
                nc.scalar.activation(
                    es_t[:, a, off:off + w],
                    ps[:, 0:w],
                    mybir.ActivationFunctionType.Exp,
                    bias=cbias[:, 0:1],
                    scale=SCALE,
                    accum_out=rrech[:, a, slot:slot + 1],
                )
                off += w
                slot += 1

        def strip(c, b, es_t):
            """Reconstruct column-strip b of the lower triangle from the
            exp'd upper strip b via one XBAR DMA transpose (symmetry)."""
            nc.sync.dma_start_transpose(
                es_t[:, b + 1:NCH, b * P:(b + 1) * P],
                es_t[:, b, (b + 1) * P:N],
            )

        def left_reduce(a, es_t, rrech):
            """Rowsum of the transposed (left-of-diagonal) part of chunk a."""
            nc.vector.tensor_reduce(
                rrech[:, a, 2:3],
                es_t[:, a, 0:a * P],
                mybir.AxisListType.X,
                mybir.AluOpType.add,
            )

        def r_chain(rrech, rbc):
            """rbc[p, i] = 1/rowsum(i) broadcast to every partition p."""
            rrec = rr2_pool.tile([P, NCH], f32, tag="rrec")
            nc.vector.tensor_reduce(
                rrec[:], rrech[:], mybir.AxisListType.X, mybir.AluOpType.add
            )
            nc.vector.reciprocal(rrec2[:, 0:NCH], rrec[:])
            nc.sync.dma_start_transpose(rt[:], rrec2[:])
            nc.sync.dma_start(rbc[0:1, :], rt[0:NCH, :])
            for h in range(2):
                nc.gpsimd.partition_broadcast(
                    rbc[:, h * 1024:(h + 1) * 1024],
                    rbc[0:1, h * 1024:(h + 1) * 1024],
                )

        # ================= Stage A =================
        # x^T is streamed twice (stream 1 feeds the c0/c1 chains, stream 2
        # the c2/c3 filler chains) so the live x window stays ~2 spans.
        with (
            tc.tile_pool(name="xs", bufs=16) as x_pool,
            tc.tile_pool(name="wq", bufs=1) as wq_pool,
        ):
            wq_sb = wq_pool.tile([P, KO, HCOLS], f16, tag="wq")
            for g in range(4):
                nc.scalar.dma_start(
                    wq_sb[:, g * 4:(g + 1) * 4, :], wq3[:, g * 4:(g + 1) * 4, :]
                )

            def xspan(s, t):
                tiles = []
                for g in range(8):
                    pc = x_pool.tile(
                        [P, 2, 512], f16, tag="xg", name=f"x{s}_{t}_{g}"
                    )
                    nc.sync.dma_start(
                        pc[:], xt3[:, g * 2:(g + 1) * 2, t * 512:(t + 1) * 512]
                    )
                    tiles.append(pc)
                return tiles

            def chain(c, t, sp):
                ps = ps_b.tile([P, 512], f32, tag="b")
                for ko in range(KO):
                    nc.tensor.matmul(
                        ps[:],
                        wq_sb[:, ko, c * P:(c + 1) * P],
                        sp[ko // 2][:, ko % 2, :],
                        start=(ko == 0),
                        stop=(ko == KO - 1),
                    )
                nc.vector.tensor_copy(qt_sb[:, c, t * 512:(t + 1) * 512], ps[:])

            # phase 1: c0/c1 chains interleaved per span (span freed after
            # its c1 chain); one span of prefetch
            x1 = {0: xspan(1, 0), 1: xspan(1, 1)}
            for t in range(4):
                if t + 2 <= 3:
                    x1[t + 2] = xspan(1, t + 2)
                chain(0, t, x1[t])
                chain(1, t, x1[t])

            es0 = es_pool.tile([P, NCH, N], f16, tag="es", name="es0")
            es1 = es_pool.tile([P, NCH, N], f16, tag="es", name="es1")
            rrech0 = rr_pool.tile([P, NCH, 3], f32, tag="rrech", name="rrech0")
            rrech1 = rr_pool.tile([P, NCH, 3], f32, tag="rrech", name="rrech1")
            nc.gpsimd.memset(rrech0[:], 0.0)
            nc.gpsimd.memset(rrech1[:], 0.0)

            # phase 2: scores/exp of heads 0,1 with c2/c3 chains as PE
            # filler (stream-2 spans loaded just in time)
            x2 = {0: xspan(2, 0)}
            fillers = []
            for t in range(4):
                fillers.append((2, t))
                fillers.append((3, t))
            fill_at = {1: 0, 5: 1, 9: 2, 13: 3, 17: 4, 21: 5, 25: 6, 29: 7}
            sq = [(0, a) for a in range(NCH)] + [(1, a) for a in range(NCH)]
            for i, (h, a) in enumerate(sq):
                es_t = es0 if h == 0 else es1
                rrech = rrech0 if h == 0 else rrech1
                scores_chunk(h, a, es_t, rrech)
                if a > 0:
                    strip(h, a - 1, es_t)
                if a >= 3:
                    left_reduce(a - 2, es_t, rrech)
                if a == NCH - 1:
                    left_reduce(NCH - 2, es_t, rrech)
                    left_reduce(NCH - 1, es_t, rrech)
                fi = fill_at.get(i)
                if fi is not None:
                    c, t = fillers[fi]
                    if c == 2 and t + 1 <= 3:
                        x2[t + 1] = xspan(2, t + 1)
                    chain(c, t, x2[t])

        # ================= Stage B =================
        with (
            tc.tile_pool(name="qn", bufs=1) as qn_pool,
            tc.tile_pool(name="yt", bufs=1) as yt_pool,
            tc.tile_pool(name="wo", bufs=1) as wo_pool,
            tc.tile_pool(name="rbc", bufs=2) as rbc_pool,
            tc.tile_pool(name="osb", bufs=3) as o_pool,
        ):
            qn_sb = qn_pool.tile([P, NCH, HCOLS], f16, tag="qn")
            yt_sb = yt_pool.tile([P, HG, N], f16, tag="yt")
            wo_sb = wo_pool.tile([P, HG, D], f16, tag="wo")

            # Qnat via XBAR transposes of QT (c,t) blocks (c0 first: PV(0)
            # only needs the c0 columns of qn as lhsT)
            for c in range(HG):
                for t in range(4):
                    nc.sync.dma_start_transpose(
                        qn_sb[:, t * 4:(t + 1) * 4, c * P:(c + 1) * P],
                        qt_sb[:, c, t * 512:(t + 1) * 512],
                    )
            nc.gpsimd.dma_start(wo_sb[:], wo3[:])

            rbc0 = rbc_pool.tile([P, N], f16, tag="rbc", name="rbc0")
            r_chain(rrech0, rbc0)
            rbc1 = rbc_pool.tile([P, N], f16, tag="rbc", name="rbc1")
            r_chain(rrech1, rbc1)

            def pv_chunk(pc, a, es_t, psy):
                for s in range(NSP):
                    nc.tensor.matmul(
                        psy[s][:],
                        qn_sb[:, a, pc * P:(pc + 1) * P],
                        es_t[:, a, s * 512:(s + 1) * 512],
                        start=(a == 0),
                        stop=(a == NCH - 1),
                    )

            def pv_evac_norm(pc, psy, rbc):
                # fused evacuate + softmax-normalize: yt = psum * (1/r)
                for s in range(NSP):
                    nc.vector.tensor_tensor(
                        yt_sb[:, pc, s * 512:(s + 1) * 512],
                        psy[s][:],
                        rbc[:, s * 512:(s + 1) * 512],
                        mybir.AluOpType.mult,
                    )

            es_tiles = {0: es0, 1: es1}
            rrechs = {0: rrech0, 1: rrech1}
            rbcs = {0: rbc0, 1: rbc1}

            # A': PV(0)+scores(2); B': PV(1)+scores(3); C': PV(2); D': PV(3)
            for it in range(4):
                pc = it
                sc = it + 2
                psy = [
                    ps_b.tile([P, 512], f32, tag="b", name=f"psy{it}_{s}")
                    for s in range(NSP)
                ]
                es_new = None
                rrech_new = None
                if sc < HG:
                    es_new = es_pool.tile(
                        [P, NCH, N], f16, tag="es", name=f"es{sc}"
                    )
                    rrech_new = rr_pool.tile(
                        [P, NCH, 3], f32, tag="rrech", name=f"rrech{sc}"
                    )
                    nc.gpsimd.memset(rrech_new[:], 0.0)
                    es_tiles[sc] = es_new
                    rrechs[sc] = rrech_new
                es_pv = es_tiles[pc]
                lr = it + 1  # head whose left reduces run this iter
                for a in range(NCH):
                    pv_chunk(pc, a, es_pv, psy)
                    if sc < HG:
                        scores_chunk(sc, a, es_new, rrech_new)
                        if a > 0:
                            strip(sc, a - 1, es_new)
                    if 2 <= lr < HG and a >= 1:
                        left_reduce(a, es_tiles[lr], rrechs[lr])
                if 2 <= lr < HG:
                    rbc_n = rbc_pool.tile(
                        [P, N], f16, tag="rbc", name=f"rbc{lr}"
                    )
                    r_chain(rrechs[lr], rbc_n)
                    rbcs[lr] = rbc_n
                pv_evac_norm(pc, psy, rbcs[pc])

            # ---------------- output projection ----------------
            oev = [0]
            for a in range(NCH):
                for d4 in range(NSP):
                    ps = ps_b.tile([P, 512], f32, tag="b")
                    for c in range(HG):
                        nc.tensor.matmul(
                            ps[:],
                            yt_sb[:, c, a * P:(a + 1) * P],
                            wo_sb[:, c, d4 * 512:(d4 + 1) * 512],
                            start=(c == 0),
                            stop=(c == HG - 1),
                        )
                    ot = o_pool.tile([P, 512], f16, tag="ot")
                    if oev[0] % 2 == 0:
                        nc.vector.tensor_copy(ot[:], ps[:])
                    else:
                        nc.scalar.copy(ot[:], ps[:])
                    eng = nc.sync if oev[0] % 2 == 0 else nc.gpsimd
                    oev[0] += 1
                    eng.dma_start(out3[:, a, d4 * 512:(d4 + 1) * 512], ot[:])

    nc.compile()
    return nc


def kernel(x, Wq, Wo, bo):
    x = np.asarray(x)
    Wq = np.asarray(Wq)
    Wo = np.asarray(Wo)
    bo = np.asarray(bo)
    B = x.shape[0]
    assert B == 2, "sharding hardcodes B=2 (core = b*4 + head_group)"
    assert x.shape == (B, N, D) and Wq.shape == (D, D) and Wo.shape == (D, D)

    if "nc" not in _CACHE:
        _CACHE["nc"] = build_nc()
    nc = _CACHE["nc"]

    xts = [np.ascontiguousarray(x[b].T).astype(np.float16) for b in range(B)]
    in_maps = []
    for core in range(8):
        b, hg = core // 4, core % 4
        in_maps.append(
            {
                "xt": xts[b],
                "wq": np.ascontiguousarray(
                    Wq[:, hg * HCOLS:(hg + 1) * HCOLS]
                ).astype(np.float16),
                "wo": np.ascontiguousarray(
                    Wo[hg * HCOLS:(hg + 1) * HCOLS, :]
                ).astype(np.float16),
            }
        )

    trace = bool(os.environ.get("KERNEL_TRACE"))
    try:
        res = run_bass_kernel_spmd(nc, in_maps, list(range(8)), trace=trace)
    except ModuleNotFoundError:
        res = run_bass_kernel_spmd(nc, in_maps, list(range(8)))
    _CACHE["last_res"] = res
    out = np.zeros((B, N, D), dtype=np.float32)
    for core in range(8):
        b = core // 4
        out[b] += res.results[core]["out"].astype(np.float32)
    out += bo.astype(np.float32)
    return out


# revision 34
# speedup vs baseline: 1.2418x; 1.2418x over previous
"""Trainium2 Bass kernel for nn_MultiHeadAttention_78864189489198.

Reference (per batch b): q = x @ Wq; qh = heads(q); S = qh qh^T / sqrt(HD)
(SYMMETRIC since q=k=v); attn = softmax(S); y = attn @ qh;
out = merge(y) @ Wo + bo.

Sharding (8 cores): core = b*4 + hg handles batch b and head-group hg
(4 heads = 512 Wq columns / Wo rows). Host sums the 4 partial outputs
per batch and adds bo (the tensor-parallel all-reduce, host-side since
I/O is host-side anyway).

Per-core dataflow (fp16 operands everywhere, fp32 psum accumulation):
  QT[hd, tok] = Wq-slice^T @ x^T chunks            (fp16 matmuls)
  Qnat = XBAR DMA transpose of QT                  (zero PE cost)
  scores: only the LOWER triangle of S is computed (S symmetric); a
    row-chunk a covers columns [0, (a+1)*128) so chunk a only needs
    token spans <= a/4 of QT -- scores/exp start as soon as the first
    Q-projection span lands.  exp via ACT (scale, bias=-C) -> es fp16,
    accum_out = partial rowsum.
  upper triangle of exp(S) reconstructed via XBAR DMA transposes of
    the exp'd lower strips (zero PE/ACT cost; one transpose per strip)
  missing rowsum parts (rowsums of the transposed right-of-diagonal
    regions) via DVE tensor_reduce
  PV: YT_unnorm[hd, tok] = sum_a Qnat_a^T @ es_a   (fp16)
  evac+normalize fused: yt = psum * (1/r broadcast row)  (one DVE op)
  out-proj: out[tok, :] = sum_c YT_c^T @ Wo_c      (fp16), fp16 output

Pipeline: stage A = x/Wq loads + Qproj (all 4 heads chained per token
span, ko-major) + scores/exp of heads 0,1 woven between spans.  Then:
A' PV(0)+scores(2), B' PV(1)+scores(3), C' PV(2), D' PV(3), out-proj.
The TimelineSim DMA model is a single serialized queue, so x is
streamed exactly once and every dependency-waiting DMA (strips) is
emitted in expected firing order on the sync queue.
"""

import os

import numpy as np

import concourse.bass as bass
import concourse.mybir as mybir
import concourse.tile as tile
from concourse import bacc
from concourse.bass_utils import run_bass_kernel_spmd
from contextlib import ExitStack

P = 128
N = 2048          # tokens
D = 2048          # model dim
KO = D // P       # 16 contraction chunks
HG = 4            # heads per core
HD = 128          # head dim
HCOLS = HG * HD   # 512 q-columns per core
NCH = N // P      # 16 token chunks
NSP = 4           # 512-token spans
SCALE = HD ** -0.5
C_BIAS = 11.0     # exp(S*SCALE - C): keeps fp16 es AND fp16 unnormalized
                  # YT in range (max scaled S ~19 -> es_max ~ e^8 = 3e3,
                  # |YT_unnorm| < ~2e4 < 65504)

f32 = mybir.dt.float32
f16 = mybir.dt.float16

_CACHE = {}


def build_nc():
    nc = bacc.Bacc("TRN2", target_bir_lowering=False, debug=False)
    xt = nc.dram_tensor("xt", [D, N], f16, kind="ExternalInput")
    wq = nc.dram_tensor("wq", [D, HCOLS], f16, kind="ExternalInput")
    wo = nc.dram_tensor("wo", [HCOLS, D], f16, kind="ExternalInput")
    out = nc.dram_tensor("out", [N, D], f16, kind="ExternalOutput")

    xt3 = xt.rearrange("(ko p) n -> p ko n", p=P)      # [128, 16, 2048]
    wq3 = wq.rearrange("(ko p) m -> p ko m", p=P)      # [128, 16, 512]
    wo3 = wo.rearrange("(c p) n -> p c n", p=P)        # [128, 4, 2048]
    out3 = out.rearrange("(a p) n -> p a n", p=P)      # [128, 16, 2048]

    with (
        nc.allow_low_precision(reason="fp16 intermediates are intentional"),
        tile.TileContext(nc) as tc,
        ExitStack() as ctx,
    ):
        const_pool = ctx.enter_context(tc.tile_pool(name="const", bufs=1))
        qt_pool = ctx.enter_context(tc.tile_pool(name="qt", bufs=1))
        qn_pool = ctx.enter_context(tc.tile_pool(name="qn", bufs=1))
        es_pool = ctx.enter_context(tc.tile_pool(name="es", bufs=2))
        rr_pool = ctx.enter_context(tc.tile_pool(name="rr", bufs=4))
        rr2_pool = ctx.enter_context(tc.tile_pool(name="rr2", bufs=2))
        rt_pool = ctx.enter_context(tc.tile_pool(name="rt", bufs=1))
        ps_s = ctx.enter_context(tc.tile_pool(name="ps_s", bufs=2, space="PSUM"))
        ps_b = ctx.enter_context(tc.tile_pool(name="ps_b", bufs=4, space="PSUM"))

        cbias = const_pool.tile([P, 1], f32, tag="cbias")
        nc.gpsimd.memset(cbias[:], -C_BIAS)
        qt_sb = qt_pool.tile([P, HG, N], f16, tag="qt")     # QT: [hd, c, tok]
        qn_sb = qn_pool.tile([P, NCH, HCOLS], f16, tag="qn")
        rrec2 = rt_pool.tile([P, P], f16, tag="rrec2")      # 1/r in cols 0:16
        rt = rt_pool.tile([P, P], f16, tag="rt")            # transposed 1/r
        nc.gpsimd.memset(rrec2[:], 0.0)

        # preload the exp table so the first real exp doesn't pay the load
        edum = const_pool.tile([P, 1], f32, tag="edum")
        nc.scalar.activation(edum[:], cbias[:], mybir.ActivationFunctionType.Exp)

        # ---------------- helpers ----------------
        def scores_chunk(c, a, es_t, rrech):
            """Lower-triangle scores row-chunk a of head c: matmuls into
            psum, exp (+rowsum accum) into es columns [0, (a+1)*128)."""
            width = (a + 1) * P
            slot = 0
            off = 0
            while off < width:
                w = min(1024, width - off)
                ps = ps_s.tile([P, 1024], f32, tag="s")
                o = 0
                while o < w:
                    mw = min(512, w - o)
                    nc.tensor.matmul(
                        ps[:, o:o + mw],
                        qt_sb[:, c, a * P:(a + 1) * P],
                        qt_sb[:, c, off + o:off + o + mw],
                        start=True,
                        stop=True,
                    )
                    o += mw
                nc.scalar.activation(
                    es_t[:, a, off:off + w],
                    ps[:, 0:w],
                    mybir.ActivationFunctionType.Exp,
                    bias=cbias[:, 0:1],
                    scale=SCALE,
                    accum_out=rrech[:, a, slot:slot + 1],
                )
                off += w
                slot += 1

        def strip(c, b, es_t):
            """Reconstruct column-strip b of the UPPER triangle from the
            exp'd lower strip of row b via one XBAR DMA transpose."""
            nc.sync.dma_start_transpose(
                es_t[:, 0:b, b * P:(b + 1) * P],
                es_t[:, b, 0:b * P],
            )

        def right_reduce(a, es_t, rrech):
            """Rowsum of the transposed (right-of-diagonal) part of row-
            chunk a.  Needs every strip b>a of this head to have landed."""
            nc.vector.tensor_reduce(
                rrech[:, a, 2:3],
                es_t[:, a, (a + 1) * P:N],
                mybir.AxisListType.X,
                mybir.AluOpType.add,
            )

        def r_chain(rrech, rbc):
            """rbc[p, i] = 1/rowsum(i) broadcast to every partition p."""
            rrec = rr2_pool.tile([P, NCH], f32, tag="rrec")
            nc.vector.tensor_reduce(
                rrec[:], rrech[:], mybir.AxisListType.X, mybir.AluOpType.add
            )
            nc.vector.reciprocal(rrec2[:, 0:NCH], rrec[:])
            nc.sync.dma_start_transpose(rt[:], rrec2[:])
            nc.sync.dma_start(rbc[0:1, :], rt[0:NCH, :])
            for h in range(2):
                nc.gpsimd.partition_broadcast(
                    rbc[:, h * 1024:(h + 1) * 1024],
                    rbc[0:1, h * 1024:(h + 1) * 1024],
                )

        def qn_transpose(c, t):
            # emitted per span (inside half_chain), so it sits in the sync
            # queue AHEAD of the later strip backlog
            nc.sync.dma_start_transpose(
                qn_sb[:, t * 4:(t + 1) * 4, c * P:(c + 1) * P],
                qt_sb[:, c, t * 512:(t + 1) * 512],
            )

        # ================= Stage A =================
        # x^T streamed ONCE; per token span one interleaved chain computes
        # all four heads ko-major (PE consumption ~6.8us/span vs ~5.8us of
        # serialized DMA per span), then the scores/exp chunks that span
        # unlocks (lower triangle: chunks 4t..4t+3 of heads 0 and 1).
        with (
            tc.tile_pool(name="xs", bufs=14) as x_pool,
            tc.tile_pool(name="wq", bufs=1) as wq_pool,
        ):
            wq_sb = wq_pool.tile([P, KO, HCOLS], f16, tag="wq")

            def xload(t, g):
                pc = x_pool.tile([P, 2, 512], f16, tag="xg", name=f"x_{t}_{g}")
                nc.sync.dma_start(
                    pc[:], xt3[:, g * 2:(g + 1) * 2, t * 512:(t + 1) * 512]
                )
                return pc

            # interleave wq groups with span-0 x so the first chain's
            # operands land together
            x1 = {}
            tiles0 = []
            for g in range(8):
                nc.scalar.dma_start(
                    wq_sb[:, g * 2:(g + 1) * 2, :],
                    wq3[:, g * 2:(g + 1) * 2, :],
                )
                tiles0.append(xload(0, g))
            x1[0] = tiles0
            x1[1] = [xload(1, g) for g in range(8)]

            es0 = es_pool.tile([P, NCH, N], f16, tag="es", name="es0")
            es1 = es_pool.tile([P, NCH, N], f16, tag="es", name="es1")
            rrech0 = rr_pool.tile([P, NCH, 4], f32, tag="rrech", name="rrech0")
            rrech1 = rr_pool.tile([P, NCH, 4], f32, tag="rrech", name="rrech1")
            nc.gpsimd.memset(rrech0[:], 0.0)
            nc.gpsimd.memset(rrech1[:], 0.0)

            def half_chain(t, sp, cs):
                """Q projection of token span t for two heads, ko-major
                (PE ~6.8us/span-half tracks the ~5.8us x DMA stream)."""
                pss = {
                    c: ps_b.tile([P, 512], f32, tag="b", name=f"qp{t}_{c}")
                    for c in cs
                }
                for ko in range(KO):
                    for c in cs:
                        nc.tensor.matmul(
                            pss[c][:],
                            wq_sb[:, ko, c * P:(c + 1) * P],
                            sp[ko // 2][:, ko % 2, :],
                            start=(ko == 0),
                            stop=(ko == KO - 1),
                        )
                for c in cs:
                    nc.vector.tensor_copy(
                        qt_sb[:, c, t * 512:(t + 1) * 512], pss[c][:]
                    )
                    qn_transpose(c, t)

            def scores_group(h, t, es_t, rrech):
                # The LAST group runs descending (15,14,13,12) so the
                # widest chunk -- the one gating PV -- exps first.
                # Head 1's strips are NOT emitted here: they would sit in
                # the serialized DMA queue ahead of head 0's late strips
                # and the qn transposes, exactly what PV(0) waits on.
                order = list(range(4 * t, 4 * t + 4))
                if t == 3:
                    order.reverse()
                for a in order:
                    scores_chunk(h, a, es_t, rrech)
                    if h == 0 and a > 0:
                        strip(h, a, es_t)

            # full span chain (PE pace ~6.8us/span tracks the ~5.8us DMA
            # stream), then the scores groups that span unlocks
            for t in range(4):
                if t + 2 <= 3:
                    x1[t + 2] = [xload(t + 2, g) for g in range(8)]
                half_chain(t, x1[t], (0, 1, 2, 3))
                scores_group(0, t, es0, rrech0)
                scores_group(1, t, es1, rrech1)

            # head-1 strips: the late ones (b>=8) are gated behind
            # exp(0,15) via a 1-element WAW copy into their destination,
            # so they cannot grab the serialized DMA queue ahead of the
            # qn-t3 transposes and head-0 g3 strips that PV(0) waits on.
            # (PV(1)/reduces(1) only need them a little later.)
            for b in range(NCH - 1, 0, -1):
                if b >= 8:
                    nc.scalar.copy(
                        es1[:, 0, b * P:b * P + 1], es0[:, NCH - 1, 0:1]
                    )
                strip(1, b, es1)
            # head-0 right reduces (need all strips of head 0; they land
            # while head 1's exps stream)
            for a in range(NCH - 2, -1, -1):
                right_reduce(a, es0, rrech0)

        # ================= Stage B =================
        with (
            tc.tile_pool(name="yt", bufs=1) as yt_pool,
            tc.tile_pool(name="wo", bufs=1) as wo_pool,
            tc.tile_pool(name="rbc", bufs=2) as rbc_pool,
            tc.tile_pool(name="osb", bufs=5) as o_pool,
        ):
            yt_sb = yt_pool.tile([P, HG, N], f16, tag="yt")
            wo_sb = wo_pool.tile([P, HG, D], f16, tag="wo")

            rbc0 = rbc_pool.tile([P, N], f16, tag="rbc", name="rbc0")
            r_chain(rrech0, rbc0)

            def load_wo_after(dep_ap):
                # Wo is only needed at out-proj, but both the tile
                # scheduler and the serialized DMA queue would otherwise
                # run its 5.8us transfer right in the congested stage-A/A'
                # boundary.  A 1-element copy reading `dep_ap` into wo_sb
                # makes the load wait until ~B' in BOTH simulators.
                nc.scalar.copy(wo_sb[0:1, 0, 0:1], dep_ap[0:1, 0:1])
                nc.gpsimd.dma_start(wo_sb[:], wo3[:])

            def pv_chunk(pc, a, es_t, psy):
                # chunks are processed DESCENDING: a==NCH-1 is the first
                # matmul of the psum accumulation group, a==0 the last
                for s in range(NSP):
                    nc.tensor.matmul(
                        psy[s][:],
                        qn_sb[:, a, pc * P:(pc + 1) * P],
                        es_t[:, a, s * 512:(s + 1) * 512],
                        start=(a == NCH - 1),
                        stop=(a == 0),
                    )

            def pv_evac_norm(pc, psy, rbc):
                # fused evacuate + softmax-normalize: yt = psum * (1/r)
                for s in range(NSP):
                    nc.vector.tensor_tensor(
                        yt_sb[:, pc, s * 512:(s + 1) * 512],
                        psy[s][:],
                        rbc[:, s * 512:(s + 1) * 512],
                        mybir.AluOpType.mult,
                    )

            es_tiles = {0: es0, 1: es1}
            rrechs = {0: rrech0, 1: rrech1}
            rbcs = {0: rbc0}
            # ACT-side right reduce for head 3's odd chunks: Copy+accum
            # into rrech slot 3, output to a scratch region that reuses an
            # rbc pool slot (allocated AFTER rbc2 so it aliases rbc1's
            # buffer, which is dead by the time the reduces run)
            scratch_box = []

            def right_reduce_act(a, es_t, rrech):
                if not scratch_box:
                    scratch_box.append(
                        rbc_pool.tile([P, N], f16, tag="rbc", name="scratch")
                    )
                w = (NCH - 1 - a) * P
                nc.scalar.activation(
                    scratch_box[0][:, 0:w],
                    es_t[:, a, (a + 1) * P:N],
                    mybir.ActivationFunctionType.Copy,
                    accum_out=rrech[:, a, 3:4],
                )

            # Every iteration processes chunks DESCENDING (15..0): PV(pc)
            # frees row 15 first, which is exactly the row whose exp the
            # next head's scores produce first, and the widest exp chunk
            # (the PV gate) is computed first.
            def iter_emit(pc, sc, lr, red_pre, lr_act_odd=False):
                psy = [
                    ps_b.tile([P, 512], f32, tag="b", name=f"psy{pc}_{s}")
                    for s in range(NSP)
                ]
                es_new = None
                rrech_new = None
                if sc < HG:
                    es_new = es_pool.tile(
                        [P, NCH, N], f16, tag="es", name=f"es{sc}"
                    )
                    rrech_new = rr_pool.tile(
                        [P, NCH, 4], f32, tag="rrech", name=f"rrech{sc}"
                    )
                    nc.gpsimd.memset(rrech_new[:], 0.0)
                    es_tiles[sc] = es_new
                    rrechs[sc] = rrech_new
                es_pv = es_tiles[pc]
                for a in range(NCH - 1, -1, -1):
                    pv_chunk(pc, a, es_pv, psy)
                    if sc < HG:
                        scores_chunk(sc, a, es_new, rrech_new)
                        if a > 0:
                            strip(sc, a, es_new)
                # ---- iteration tail ----
                reds = []
                if lr is not None:
                    if lr_act_odd:
                        # heads 2/3: odd chunks reduce on ACT (it has holes
                        # during the exp stream and is free afterwards),
                        # even chunks on DVE
                        for a in range(NCH - 2, -1, -1):
                            if a % 2 == 1:
                                right_reduce_act(a, es_tiles[lr], rrechs[lr])
                        reds = [a for a in range(NCH - 2, -1, -1) if a % 2 == 0]
                    else:
                        reds = list(range(NCH - 2, -1, -1))
                for a in reds[:red_pre]:
                    right_reduce(a, es_tiles[lr], rrechs[lr])
                pv_evac_norm(pc, psy, rbcs[pc])
                for a in reds[red_pre:]:
                    right_reduce(a, es_tiles[lr], rrechs[lr])
                if lr is not None:
                    rbc_n = rbc_pool.tile(
                        [P, N], f16, tag="rbc", name=f"rbc{lr}"
                    )
                    r_chain(rrechs[lr], rbc_n)
                    rbcs[lr] = rbc_n

            iter_emit(0, 2, 1, red_pre=3)            # A'
            load_wo_after(rbcs[1])
            iter_emit(1, 3, 2, red_pre=5)            # B'
            iter_emit(2, 4, 3, red_pre=2, lr_act_odd=True)   # C'
            iter_emit(3, 4, None, red_pre=0)         # D'

            # ---------------- output projection ----------------
            oev = [0]
            for a in range(NCH):
                for d4 in range(NSP):
                    ps = ps_b.tile([P, 512], f32, tag="b")
                    for c in range(HG):
                        nc.tensor.matmul(
                            ps[:],
                            yt_sb[:, c, a * P:(a + 1) * P],
                            wo_sb[:, c, d4 * 512:(d4 + 1) * 512],
                            start=(c == 0),
                            stop=(c == HG - 1),
                        )
                    ot = o_pool.tile([P, 512], f16, tag="ot")
                    if oev[0] % 2 == 0:
                        # DVE evacuates; ACT's queue issues the DMA (the
                        # evac is long done when the issuing queue reaches
                        # it, so no head-of-line wait on either queue)
                        nc.vector.tensor_copy(ot[:], ps[:])
                        dma_eng = nc.scalar
                    else:
                        nc.scalar.copy(ot[:], ps[:])
                        dma_eng = nc.sync
                    oev[0] += 1
                    dma_eng.dma_start(out3[:, a, d4 * 512:(d4 + 1) * 512], ot[:])

    nc.compile()
    return nc


def kernel(x, Wq, Wo, bo):
    x = np.asarray(x)
    Wq = np.asarray(Wq)
    Wo = np.asarray(Wo)
    bo = np.asarray(bo)
    B = x.shape[0]
    assert B == 2, "sharding hardcodes B=2 (core = b*4 + head_group)"
    assert x.shape == (B, N, D) and Wq.shape == (D, D) and Wo.shape == (D, D)

    if "nc" not in _CACHE:
        _CACHE["nc"] = build_nc()
    nc = _CACHE["nc"]

    xts = [np.ascontiguousarray(x[b].T).astype(np.float16) for b in range(B)]
    in_maps = []
    for core in range(8):
        b, hg = core // 4, core % 4
        in_maps.append(
            {
                "xt": xts[b],
                "wq": np.ascontiguousarray(
                    Wq[:, hg * HCOLS:(hg + 1) * HCOLS]
                ).astype(np.float16),
                "wo": np.ascontiguousarray(
                    Wo[hg * HCOLS:(hg + 1) * HCOLS, :]
                ).astype(np.float16),
            }
        )

    trace = bool(os.environ.get("KERNEL_TRACE"))
    try:
        res = run_bass_kernel_spmd(nc, in_maps, list(range(8)), trace=trace)
    except ModuleNotFoundError:
        res = run_bass_kernel_spmd(nc, in_maps, list(range(8)))
    _CACHE["last_res"] = res
    out = np.zeros((B, N, D), dtype=np.float32)
    for core in range(8):
        b = core // 4
        out[b] += res.results[core]["out"].astype(np.float32)
    out += bo.astype(np.float32)
    return out


# revision 47
# speedup vs baseline: 1.2930x; 1.0413x over previous
"""Trainium2 Bass kernel for nn_MultiHeadAttention_78864189489198.

Reference (per batch b): q = x @ Wq; qh = heads(q); S = qh qh^T / sqrt(HD)
(SYMMETRIC since q=k=v); attn = softmax(S); y = attn @ qh;
out = merge(y) @ Wo + bo.

Sharding (8 cores): core = b*4 + hg handles batch b and head-group hg
(4 heads = 512 Wq columns / Wo rows). Host sums the 4 partial outputs
per batch and adds bo (the tensor-parallel all-reduce, host-side since
I/O is host-side anyway).

Per-core dataflow (fp16 operands everywhere, fp32 psum accumulation):
  QT[hd, tok] = Wq-slice^T @ x^T chunks            (fp16 matmuls)
  Qnat = XBAR DMA transpose of QT                  (zero PE cost)
  scores: only the LOWER triangle of S is computed (S symmetric); a
    row-chunk a covers columns [0, (a+1)*128) so chunk a only needs
    token spans <= a/4 of QT -- scores/exp start as soon as the first
    Q-projection span lands.  exp via ACT (scale, bias=-C) -> es fp16,
    accum_out = partial rowsum.
  upper triangle of exp(S) reconstructed via XBAR DMA transposes of
    the exp'd lower strips (zero PE/ACT cost; one transpose per strip)
  missing rowsum parts (rowsums of the transposed right-of-diagonal
    regions) via DVE tensor_reduce
  PV: YT_unnorm[hd, tok] = sum_a Qnat_a^T @ es_a   (fp16)
  evac+normalize fused: yt = psum * (1/r broadcast row)  (one DVE op)
  out-proj: out[tok, :] = sum_c YT_c^T @ Wo_c      (fp16), fp16 output

Pipeline: stage A = x/Wq loads + Qproj (all 4 heads chained per token
span, ko-major) + scores/exp of heads 0,1 woven between spans.  Then:
A' PV(0)+scores(2), B' PV(1)+scores(3), C' PV(2), D' PV(3), out-proj.
The TimelineSim DMA model is a single serialized queue, so x is
streamed exactly once and every dependency-waiting DMA (strips) is
emitted in expected firing order on the sync queue.
"""

import os

import numpy as np

import concourse.bass as bass
import concourse.mybir as mybir
import concourse.tile as tile
from concourse import bacc
from concourse.bass_utils import run_bass_kernel_spmd
from contextlib import ExitStack

P = 128
N = 2048          # tokens
D = 2048          # model dim
KO = D // P       # 16 contraction chunks
HG = 4            # heads per core
HD = 128          # head dim
HCOLS = HG * HD   # 512 q-columns per core
NCH = N // P      # 16 token chunks
NSP = 4           # 512-token spans
SCALE = HD ** -0.5
C_BIAS = 11.0     # exp(S*SCALE - C): keeps fp16 es AND fp16 unnormalized
                  # YT in range (max scaled S ~19 -> es_max ~ e^8 = 3e3,
                  # |YT_unnorm| < ~2e4 < 65504)

f32 = mybir.dt.float32
f16 = mybir.dt.float16

_CACHE = {}


def build_nc():
    nc = bacc.Bacc("TRN2", target_bir_lowering=False, debug=False)
    xt = nc.dram_tensor("xt", [D, N], f16, kind="ExternalInput")
    wq = nc.dram_tensor("wq", [D, HCOLS], f16, kind="ExternalInput")
    wo = nc.dram_tensor("wo", [HCOLS, D], f16, kind="ExternalInput")
    out = nc.dram_tensor("out", [N, D], f16, kind="ExternalOutput")

    xt3 = xt.rearrange("(ko p) n -> p ko n", p=P)      # [128, 16, 2048]
    wq3 = wq.rearrange("(ko p) m -> p ko m", p=P)      # [128, 16, 512]
    wo3 = wo.rearrange("(c p) n -> p c n", p=P)        # [128, 4, 2048]
    out3 = out.rearrange("(a p) n -> p a n", p=P)      # [128, 16, 2048]

    with (
        nc.allow_low_precision(reason="fp16 intermediates are intentional"),
        tile.TileContext(nc) as tc,
        ExitStack() as ctx,
    ):
        const_pool = ctx.enter_context(tc.tile_pool(name="const", bufs=1))
        qt_pool = ctx.enter_context(tc.tile_pool(name="qt", bufs=1))
        qn_pool = ctx.enter_context(tc.tile_pool(name="qn", bufs=1))
        es_pool = ctx.enter_context(tc.tile_pool(name="es", bufs=2))
        rr_pool = ctx.enter_context(tc.tile_pool(name="rr", bufs=4))
        rr2_pool = ctx.enter_context(tc.tile_pool(name="rr2", bufs=2))
        rt_pool = ctx.enter_context(tc.tile_pool(name="rt", bufs=1))
        ps_s = ctx.enter_context(tc.tile_pool(name="ps_s", bufs=2, space="PSUM"))
        ps_b = ctx.enter_context(tc.tile_pool(name="ps_b", bufs=4, space="PSUM"))

        cbias = const_pool.tile([P, 1], f32, tag="cbias")
        nc.gpsimd.memset(cbias[:], -C_BIAS)
        qt_sb = qt_pool.tile([P, HG, N], f16, tag="qt")     # QT: [hd, c, tok]
        qn_sb = qn_pool.tile([P, NCH, HCOLS], f16, tag="qn")
        rrec2 = rt_pool.tile([P, P], f16, tag="rrec2")      # 1/r in cols 0:16
        rt = rt_pool.tile([P, P], f16, tag="rt")            # transposed 1/r
        nc.gpsimd.memset(rrec2[:], 0.0)

        # preload the exp table so the first real exp doesn't pay the load
        edum = const_pool.tile([P, 1], f32, tag="edum")
        nc.scalar.activation(edum[:], cbias[:], mybir.ActivationFunctionType.Exp)

        # ---------------- helpers ----------------
        def scores_chunk(c, a, es_t, rrech):
            """Lower-triangle scores row-chunk a of head c: matmuls into
            psum, exp (+rowsum accum) into es columns [0, (a+1)*128)."""
            width = (a + 1) * P
            slot = 0
            off = 0
            while off < width:
                w = min(1024, width - off)
                ps = ps_s.tile([P, 1024], f32, tag="s")
                o = 0
                while o < w:
                    mw = min(512, w - o)
                    nc.tensor.matmul(
                        ps[:, o:o + mw],
                        qt_sb[:, c, a * P:(a + 1) * P],
                        qt_sb[:, c, off + o:off + o + mw],
                        start=True,
                        stop=True,
                    )
                    o += mw
                nc.scalar.activation(
                    es_t[:, a, off:off + w],
                    ps[:, 0:w],
                    mybir.ActivationFunctionType.Exp,
                    bias=cbias[:, 0:1],
                    scale=SCALE,
                    accum_out=rrech[:, a, slot:slot + 1],
                )
                off += w
                slot += 1

        def strip(c, b, es_t):
            """Reconstruct column-strip b of the UPPER triangle from the
            exp'd lower strip of row b via one XBAR DMA transpose."""
            nc.sync.dma_start_transpose(
                es_t[:, 0:b, b * P:(b + 1) * P],
                es_t[:, b, 0:b * P],
            )

        def right_reduce(a, es_t, rrech, gate=None):
            """Rowsum of the transposed (right-of-diagonal) part of row-
            chunk a.  Needs every strip b>a of this head to have landed.
            `gate` (an AP): a 1-element WAW copy makes the reduce fire
            only after that value is written -- used to keep the reduce
            backlog BEHIND the boundary-critical psum evacuations on the
            in-order DVE queue."""
            if gate is not None:
                nc.gpsimd.tensor_copy(rrech[0:1, a, 2:3], gate)
            nc.vector.tensor_reduce(
                rrech[:, a, 2:3],
                es_t[:, a, (a + 1) * P:N],
                mybir.AxisListType.X,
                mybir.AluOpType.add,
            )

        def r_chain(rrech, rbc):
            """rbc[p, i] = 1/rowsum(i) broadcast to every partition p."""
            rrec = rr2_pool.tile([P, NCH], f32, tag="rrec")
            nc.vector.tensor_reduce(
                rrec[:], rrech[:], mybir.AxisListType.X, mybir.AluOpType.add
            )
            nc.vector.reciprocal(rrec2[:, 0:NCH], rrec[:])
            nc.sync.dma_start_transpose(rt[:], rrec2[:])
            nc.sync.dma_start(rbc[0:1, :], rt[0:NCH, :])
            for h in range(2):
                nc.gpsimd.partition_broadcast(
                    rbc[:, h * 1024:(h + 1) * 1024],
                    rbc[0:1, h * 1024:(h + 1) * 1024],
                )

        def qn_transpose(c, t):
            # emitted per span (inside half_chain), so it sits in the sync
            # queue AHEAD of the later strip backlog
            nc.sync.dma_start_transpose(
                qn_sb[:, t * 4:(t + 1) * 4, c * P:(c + 1) * P],
                qt_sb[:, c, t * 512:(t + 1) * 512],
            )

        # ================= Stage A =================
        # x^T streamed ONCE; per token span one interleaved chain computes
        # all four heads ko-major (PE consumption ~6.8us/span vs ~5.8us of
        # serialized DMA per span), then the scores/exp chunks that span
        # unlocks (lower triangle: chunks 4t..4t+3 of heads 0 and 1).
        with (
            tc.tile_pool(name="xs", bufs=14) as x_pool,
            tc.tile_pool(name="wq", bufs=1) as wq_pool,
        ):
            wq_sb = wq_pool.tile([P, KO, HCOLS], f16, tag="wq")

            def xload(t, g):
                pc = x_pool.tile([P, 2, 512], f16, tag="xg", name=f"x_{t}_{g}")
                nc.sync.dma_start(
                    pc[:], xt3[:, g * 2:(g + 1) * 2, t * 512:(t + 1) * 512]
                )
                return pc

            # interleave wq groups with span-0 x so the first chain's
            # operands land together
            x1 = {}
            tiles0 = []
            for g in range(8):
                nc.scalar.dma_start(
                    wq_sb[:, g * 2:(g + 1) * 2, :],
                    wq3[:, g * 2:(g + 1) * 2, :],
                )
                tiles0.append(xload(0, g))
            x1[0] = tiles0
            x1[1] = [xload(1, g) for g in range(8)]

            es0 = es_pool.tile([P, NCH, N], f16, tag="es", name="es0")
            es1 = es_pool.tile([P, NCH, N], f16, tag="es", name="es1")
            rrech0 = rr_pool.tile([P, NCH, 4], f32, tag="rrech", name="rrech0")
            rrech1 = rr_pool.tile([P, NCH, 4], f32, tag="rrech", name="rrech1")
            nc.gpsimd.memset(rrech0[:], 0.0)
            nc.gpsimd.memset(rrech1[:], 0.0)

            def half_chain(t, sp, cs):
                """Q projection of token span t for two heads, ko-major
                (PE ~6.8us/span-half tracks the ~5.8us x DMA stream)."""
                pss = {
                    c: ps_b.tile([P, 512], f32, tag="b", name=f"qp{t}_{c}")
                    for c in cs
                }
                for ko in range(KO):
                    for c in cs:
                        nc.tensor.matmul(
                            pss[c][:],
                            wq_sb[:, ko, c * P:(c + 1) * P],
                            sp[ko // 2][:, ko % 2, :],
                            start=(ko == 0),
                            stop=(ko == KO - 1),
                        )
                for c in cs:
                    nc.vector.tensor_copy(
                        qt_sb[:, c, t * 512:(t + 1) * 512], pss[c][:]
                    )
                    qn_transpose(c, t)

            def scores_group(h, t, es_t, rrech):
                # The LAST group runs descending (15,14,13,12) so the
                # widest chunk -- the one gating PV -- exps first.
                # Head 1's strips are NOT emitted here: they would sit in
                # the serialized DMA queue ahead of head 0's late strips
                # and the qn transposes, exactly what PV(0) waits on.
                order = list(range(4 * t, 4 * t + 4))
                if t == 3:
                    order.reverse()
                for a in order:
                    scores_chunk(h, a, es_t, rrech)
                    if h == 0 and a > 0:
                        strip(h, a, es_t)

            # full span chain (PE pace ~6.8us/span tracks the ~5.8us DMA
            # stream), then the scores groups that span unlocks
            for t in range(4):
                if t + 2 <= 3:
                    x1[t + 2] = [xload(t + 2, g) for g in range(8)]
                half_chain(t, x1[t], (0, 1, 2, 3))
                scores_group(0, t, es0, rrech0)
                scores_group(1, t, es1, rrech1)

            # head-1 strips: the late ones (b>=8) are gated behind
            # exp(0,15) via a 1-element WAW copy into their destination,
            # so they cannot grab the serialized DMA queue ahead of the
            # qn-t3 transposes and head-0 g3 strips that PV(0) waits on.
            # (PV(1)/reduces(1) only need them a little later.)
            for b in range(NCH - 1, 0, -1):
                if b >= 8:
                    nc.scalar.copy(
                        es1[:, 0, b * P:b * P + 1], es0[:, NCH - 1, 0:1]
                    )
                strip(1, b, es1)
            # head-0 right reduces (need all strips of head 0; they land
            # while head 1's exps stream)
            for a in range(NCH - 2, -1, -1):
                right_reduce(a, es0, rrech0)

        # ================= Stage B =================
        with (
            tc.tile_pool(name="yt", bufs=1) as yt_pool,
            tc.tile_pool(name="wo", bufs=1) as wo_pool,
            tc.tile_pool(name="rbc", bufs=2) as rbc_pool,
            tc.tile_pool(name="osb", bufs=5) as o_pool,
        ):
            yt_sb = yt_pool.tile([P, HG, N], f16, tag="yt")
            wo_sb = wo_pool.tile([P, HG, D], f16, tag="wo")

            rbc0 = rbc_pool.tile([P, N], f16, tag="rbc", name="rbc0")
            r_chain(rrech0, rbc0)

            def load_wo_after(dep_ap):
                # Wo is only needed at out-proj, but both the tile
                # scheduler and the serialized DMA queue would otherwise
                # run its 5.8us transfer right in the congested stage-A/A'
                # boundary.  A 1-element copy reading `dep_ap` into wo_sb
                # makes the load wait until ~B' in BOTH simulators.
                nc.scalar.copy(wo_sb[0:1, 0, 0:1], dep_ap[0:1, 0:1])
                nc.gpsimd.dma_start(wo_sb[:], wo3[:])

            def pv_chunk(pc, a, es_t, psy):
                # chunks are processed DESCENDING: a==NCH-1 is the first
                # matmul of the psum accumulation group, a==0 the last
                for s in range(NSP):
                    nc.tensor.matmul(
                        psy[s][:],
                        qn_sb[:, a, pc * P:(pc + 1) * P],
                        es_t[:, a, s * 512:(s + 1) * 512],
                        start=(a == NCH - 1),
                        stop=(a == 0),
                    )

            def pv_evac(pc, psy):
                # plain evacuate: psum release never waits on the r-chain
                for s in range(NSP):
                    nc.vector.tensor_copy(
                        yt_sb[:, pc, s * 512:(s + 1) * 512], psy[s][:]
                    )

            def norm(pc, rbc):
                # in-place softmax-normalize (all fp16 SBUF -> DVE 2x mode)
                for s in range(NSP):
                    nc.vector.tensor_tensor(
                        yt_sb[:, pc, s * 512:(s + 1) * 512],
                        yt_sb[:, pc, s * 512:(s + 1) * 512],
                        rbc[:, s * 512:(s + 1) * 512],
                        mybir.AluOpType.mult,
                    )

            es_tiles = {0: es0, 1: es1}
            rrechs = {0: rrech0, 1: rrech1}
            rbcs = {0: rbc0}
            # ACT-side right reduce for head 3's odd chunks: Copy+accum
            # into rrech slot 3, output to a scratch region that reuses an
            # rbc pool slot (allocated AFTER rbc2 so it aliases rbc1's
            # buffer, which is dead by the time the reduces run)
            scratch_box = []

            def right_reduce_act(a, es_t, rrech):
                if not scratch_box:
                    scratch_box.append(
                        rbc_pool.tile([P, N], f16, tag="rbc", name="scratch")
                    )
                w = (NCH - 1 - a) * P
                nc.scalar.activation(
                    scratch_box[0][:, 0:w],
                    es_t[:, a, (a + 1) * P:N],
                    mybir.ActivationFunctionType.Copy,
                    accum_out=rrech[:, a, 3:4],
                )

            # Every iteration processes chunks DESCENDING (15..0): PV(pc)
            # frees row 15 first, which is exactly the row whose exp the
            # next head's scores produce first, and the widest exp chunk
            # (the PV gate) is computed first.
            def iter_emit(pc, sc, lr, red_pre, lr_act_odd=False):
                psy = [
                    ps_b.tile([P, 512], f32, tag="b", name=f"psy{pc}_{s}")
                    for s in range(NSP)
                ]
                es_new = None
                rrech_new = None
                if sc < HG:
                    es_new = es_pool.tile(
                        [P, NCH, N], f16, tag="es", name=f"es{sc}"
                    )
                    rrech_new = rr_pool.tile(
                        [P, NCH, 4], f32, tag="rrech", name=f"rrech{sc}"
                    )
                    nc.gpsimd.memset(rrech_new[:], 0.0)
                    es_tiles[sc] = es_new
                    rrechs[sc] = rrech_new
                es_pv = es_tiles[pc]
                for a in range(NCH - 1, -1, -1):
                    pv_chunk(pc, a, es_pv, psy)
                    if sc < HG:
                        scores_chunk(sc, a, es_new, rrech_new)
                        if a > 0:
                            strip(sc, a, es_new)
                # ---- iteration tail ----
                reds = []
                if lr is not None:
                    if lr_act_odd:
                        # heads 2/3: odd chunks reduce on ACT (it has holes
                        # during the exp stream and is free afterwards),
                        # even chunks on DVE
                        for a in range(NCH - 2, -1, -1):
                            if a % 2 == 1:
                                right_reduce_act(a, es_tiles[lr], rrechs[lr])
                        reds = [a for a in range(NCH - 2, -1, -1) if a % 2 == 0]
                    else:
                        reds = list(range(NCH - 2, -1, -1))
                for a in reds[:red_pre]:
                    right_reduce(a, es_tiles[lr], rrechs[lr])
                pv_evac(pc, psy)
                if pc >= 1:
                    norm(pc - 1, rbcs[pc - 1])
                for a in reds[red_pre:]:
                    right_reduce(a, es_tiles[lr], rrechs[lr])
                if lr is not None:
                    rbc_n = rbc_pool.tile(
                        [P, N], f16, tag="rbc", name=f"rbc{lr}"
                    )
                    r_chain(rrechs[lr], rbc_n)
                    rbcs[lr] = rbc_n

            iter_emit(0, 2, 1, red_pre=3)            # A'
            load_wo_after(rbcs[1])
            iter_emit(1, 3, 2, red_pre=5)            # B'
            # C'+D' merged: PV(2) and PV(3) interleaved.  No scores run
            # here, so PV(3) accumulates in the otherwise-idle ps_s banks:
            # no iteration boundary, and PV(3) never waits on evac(2).
            psy2 = [
                ps_b.tile([P, 512], f32, tag="b", name=f"psy2_{s}")
                for s in range(NSP)
            ]
            ps3a = ps_s.tile([P, 1024], f32, tag="s", name="ps3a")
            ps3b = ps_s.tile([P, 1024], f32, tag="s", name="ps3b")
            psy3 = [ps3a[:, 0:512], ps3a[:, 512:1024],
                    ps3b[:, 0:512], ps3b[:, 512:1024]]
            norm(1, rbcs[1])
            # head-3 odd-chunk reduces on ACT (free once exp(3) ends)
            for a in range(NCH - 2, -1, -1):
                if a % 2 == 1:
                    right_reduce_act(a, es_tiles[3], rrechs[3])

            def pv3_chunk(a):
                for s in range(NSP):
                    nc.tensor.matmul(
                        psy3[s],
                        qn_sb[:, a, 3 * P:4 * P],
                        es_tiles[3][:, a, s * 512:(s + 1) * 512],
                        start=(a == NCH - 1),
                        stop=(a == 0),
                    )

            # PV(3)'s psum tiles reuse the ps_s banks, which only free once
            # the exp(3) stream fully drains -- so the first 8 chunks run
            # PV(2) only (in-order PE must not park on PV(3,15)), then
            # PV(3) catches up at two chunks per PV(2) chunk.
            for a in range(NCH - 1, 7, -1):
                pv_chunk(2, a, es_tiles[2], psy2)
                if a % 2 == 0:
                    right_reduce(a, es_tiles[3], rrechs[3])
            pv3 = NCH - 1
            for a in range(7, -1, -1):
                pv_chunk(2, a, es_tiles[2], psy2)
                pv3_chunk(pv3)
                pv3_chunk(pv3 - 1)
                pv3 -= 2
                if a % 2 == 0:
                    right_reduce(a, es_tiles[3], rrechs[3])
            pv_evac(2, psy2)
            rbc3 = rbc_pool.tile([P, N], f16, tag="rbc", name="rbc3")
            r_chain(rrechs[3], rbc3)
            rbcs[3] = rbc3
            for s in range(NSP):
                nc.vector.tensor_copy(
                    yt_sb[:, 3, s * 512:(s + 1) * 512], psy3[s]
                )
            norm(2, rbcs[2])
            norm(3, rbcs[3])

            # ---------------- output projection ----------------
            oev = [0]
            for a in range(NCH):
                for d4 in range(NSP):
                    ps = ps_b.tile([P, 512], f32, tag="b")
                    for c in range(HG):
                        nc.tensor.matmul(
                            ps[:],
                            yt_sb[:, c, a * P:(a + 1) * P],
                            wo_sb[:, c, d4 * 512:(d4 + 1) * 512],
                            start=(c == 0),
                            stop=(c == HG - 1),
                        )
                    ot = o_pool.tile([P, 512], f16, tag="ot")
                    if oev[0] % 2 == 0:
                        # DVE evacuates; ACT's queue issues the DMA (the
                        # evac is long done when the issuing queue reaches
                        # it, so no head-of-line wait on either queue)
                        nc.vector.tensor_copy(ot[:], ps[:])
                        dma_eng = nc.scalar
                    else:
                        nc.scalar.copy(ot[:], ps[:])
                        dma_eng = nc.sync
                    oev[0] += 1
                    dma_eng.dma_start(out3[:, a, d4 * 512:(d4 + 1) * 512], ot[:])

    nc.compile()
    return nc


def kernel(x, Wq, Wo, bo):
    x = np.asarray(x)
    Wq = np.asarray(Wq)
    Wo = np.asarray(Wo)
    bo = np.asarray(bo)
    B = x.shape[0]
    assert B == 2, "sharding hardcodes B=2 (core = b*4 + head_group)"
    assert x.shape == (B, N, D) and Wq.shape == (D, D) and Wo.shape == (D, D)

    if "nc" not in _CACHE:
        _CACHE["nc"] = build_nc()
    nc = _CACHE["nc"]

    xts = [np.ascontiguousarray(x[b].T).astype(np.float16) for b in range(B)]
    in_maps = []
    for core in range(8):
        b, hg = core // 4, core % 4
        in_maps.append(
            {
                "xt": xts[b],
                "wq": np.ascontiguousarray(
                    Wq[:, hg * HCOLS:(hg + 1) * HCOLS]
                ).astype(np.float16),
                "wo": np.ascontiguousarray(
                    Wo[hg * HCOLS:(hg + 1) * HCOLS, :]
                ).astype(np.float16),
            }
        )

    trace = bool(os.environ.get("KERNEL_TRACE"))
    try:
        res = run_bass_kernel_spmd(nc, in_maps, list(range(8)), trace=trace)
    except ModuleNotFoundError:
        res = run_bass_kernel_spmd(nc, in_maps, list(range(8)))
    _CACHE["last_res"] = res
    out = np.zeros((B, N, D), dtype=np.float32)
    for core in range(8):
        b = core // 4
        out[b] += res.results[core]["out"].astype(np.float32)
    out += bo.astype(np.float32)
    return out
